# revision 38
# baseline (speedup 1.0000x reference)
"""Trainium2 Bass kernel for nn_Network_79061757985000 (dense_mlp).

  h = x @ binarize(W1).T          [65536, 300]
  h = batchnorm(h, gamma1, beta1)
  o = h @ binarize(W2).T          [65536, 10]
  out = batchnorm(o, gamma2, beta2)

Strategy (8 NeuronCores, pure data parallelism over the batch):
  - Host pre-pads x 784->896 and casts fp32->fp16 (the same RNE cast the
    DMA engine would apply); host also binarizes/transposes/packs the tiny
    weight matrices into their exact SBUF layouts so the device does three
    small DMAs instead of a DMA+sign+transpose chain.
  - Each core streams its 8192 rows as 8 chunks of 1024 rows in
    [128p, 8 rows, 896] layout (14.3KB contiguous per partition per chunk).
  - Slab g of a chunk holds batch rows {8p+g}; slabs are transposed to
    [d, b] layout, split between the DMA xbar (sync queue) and PE
    transposes, pipelined one chunk ahead of the matmuls.
  - Layer 1: out[kc<=128, 512] = W1bT[d,kc].T @ xT[d, 512] (fp16 operands,
    fp32 PSUM accumulation); loops ordered (ci, j, group-pair) so
    consecutive matmuls share the stationary operand.
  - BN1 batch stats via DVE bn_stats on the PSUM tiles; per-core partials
    AllGather'd and reduced locally.
  - BN1 + layer 2 folded: o' = (h * a1) @ W2b.T with a1 =
    gamma1*rsqrt(var+eps); remaining BN1 affine constants are
    batch-constant and cancel inside BN2.
  - Layer 2: out[10, 512] = W2aT[k,10].T @ hT[k, 512] (fp16), groups in
    blocks of 4 with ci outer to reuse stationaries.
  - BN2 stats AllGather'd; final affine applied in [10, b] layout, tiles
    PE-transposed back to [b, 10] and stored blocked: partition p writes
    DRAM rows [64p, 64p+64) as one contiguous 2560B run. The host
    unpermutes rows of the returned array (free).

The scale factors of the binarized matmuls cancel inside the batchnorms,
so fp16 inputs only contribute ~3e-4 relative error.
"""
import sys

sys.path.insert(0, "/opt/trn_rl_repo")

import numpy as np

import concourse.bass as bass
import concourse.tile as tile
from concourse import bacc, masks, mybir
from concourse import bass_utils

N_CORES = 8
B_FULL = 65536
BC = B_FULL // N_CORES          # 8192 rows per core
D = 784                         # input features
ND = 7                          # d-chunks of 128 (784 -> 896 padded)
DPAD = ND * 128                 # 896
H = 300                         # hidden features
KCH = [(0, 128, 0), (128, 128, 128), (256, 44, 256)]  # (k0, kc, off)
WCOLS = 304                     # packed w1bT column count (128+128+48)
O = 10                          # output features
EPS = 1e-5
CAST_ROWS = 1024                # rows per chunk
NCHUNK = BC // CAST_ROWS        # 8
SLABS = CAST_ROWS // 128        # 8 slabs of 128 rows
GW = 512                        # moving free dim per matmul group
NGRP = BC // GW                 # 16 groups per core
GBLK = 4                        # L2 groups per stationary block
XBAR_SYNC = (0, 1, 2)           # slabs transposed via xbar on sync queue
XBAR_SCAL = ()                  # slabs transposed via xbar on scalar queue
XIO_BUFS = 4                    # x chunk buffers in flight

f32 = mybir.dt.float32
f16 = mybir.dt.float16
AF = mybir.ActivationFunctionType
ALU = mybir.AluOpType


def _emit(nc, tc, io, P, ranks, debug, l1_only=False):
    """Emit one full forward pass."""
    pp, xio, xTp = P["pp"], P["xio"], P["xTp"]
    ps_h, ps_t, dram = P["ps_h"], P["ps_t"], P["dram"]
    ps_o = ps_h

    # identity early (cheap gpsimd ops; unblocks the PE transpose path)
    i128_16 = pp.tile([128, 128], f16, tag="i128_16", name="i128_16")
    masks.make_identity(nc, i128_16[:])

    # ---------------- x chunk loads (gpsimd queue, fp16, padded) --------
    def load_chunk(c, split=False):
        t = xio.tile([128, SLABS, DPAD], f16, tag="x16", name="x16")
        src = io["x"].ap()[c * CAST_ROWS:(c + 1) * CAST_ROWS, :] \
            .rearrange("(p g) d -> p (g d)", p=128)
        dst = t[:].rearrange("p g d -> p (g d)")
        eng = nc.gpsimd
        if split:
            hw = SLABS // 2 * DPAD
            eng.dma_start(dst[:, 0:hw], src[:, 0:hw])
            eng.dma_start(dst[:, hw:2 * hw], src[:, hw:2 * hw])
        else:
            eng.dma_start(dst, src)
        return t

    # ---------------- host-packed weights first (gate the first matmuls) -
    w1w = pp.tile([128, ND, WCOLS], f16, tag="w1w", name="w1w")
    nc.scalar.dma_start(w1w[:].rearrange("p a b -> p (a b)"), io["w1bT"].ap())
    w2w = pp.tile([128, 3 * O], f16, tag="w2w", name="w2w")
    nc.scalar.dma_start(w2w[:], io["w2bT"].ap())
    prm = pp.tile([128, 8], f32, tag="prm", name="prm")
    nc.scalar.dma_start(prm[:], io["prm"].ap())

    x16s = [load_chunk(0, split=True)] + \
           [load_chunk(c) for c in range(1, XIO_BUFS - 1)]

    # ---------------- persistent state ----------------
    hT = [pp.tile([128, BC], f16, tag=f"hT{ci}", name=f"hT{ci}")
          for ci in range(3)]
    bst = pp.tile([128, 3, NGRP, 6], f32, tag="bst", name="bst")
    oT = pp.tile([O, BC], f32, tag="oT", name="oT")
    bst2 = pp.tile([O, NGRP, 6], f32, tag="bst2", name="bst2")

    # ---------------- layer 1 (transposes one chunk ahead) --------------
    def emit_transposes(c, x16):
        # [128 b, 8 g, 896 d] -> [128 d, 8 g, 7 j, 128 b]
        xT2 = xTp.tile([128, SLABS, ND, 128], f16, tag="xT2", name="xT2")
        for g in range(SLABS):
            if g in XBAR_SYNC:
                nc.sync.dma_start(xT2[:, g:g + 1, :, :], x16[:, g:g + 1, :],
                                  transpose=True)
            elif g in XBAR_SCAL:
                nc.scalar.dma_start(xT2[:, g:g + 1, :, :], x16[:, g:g + 1, :],
                                    transpose=True)
            else:
                tpx = ps_t.tile([128, ND, 128], f16, tag="otps", name="tpx")
                for j in range(ND):
                    nc.tensor.transpose(
                        tpx[:, j, :], x16[:, g:g + 1, 128 * j:128 * (j + 1)],
                        i128_16[:])
                if g % 2 == 1:
                    nc.scalar.copy(xT2[:, g, :, :], tpx[:])
                else:
                    nc.vector.tensor_copy(xT2[:, g, :, :], tpx[:])
        return xT2

    def emit_matmuls(c, xT2):
        ng = CAST_ROWS // GW                      # 2 groups per chunk
        for ci, (k0, kc, off) in enumerate(KCH):
            hps = [ps_h.tile([128, GW], f32, tag="hps", name="hps")
                   for _ in range(ng)]
            for j in range(ND):
                for g2 in range(ng):
                    nc.tensor.matmul(
                        hps[g2][0:kc, :],
                        w1w[:, j:j + 1, off:off + kc],
                        xT2[:, 4 * g2:4 * (g2 + 1), j:j + 1, :],
                        start=(j == 0), stop=(j == ND - 1))
            for g2 in range(ng):
                g = c * ng + g2
                nc.scalar.copy(hT[ci][0:kc, GW * g:GW * (g + 1)],
                               hps[g2][0:kc, :])
                nc.vector.bn_stats(bst[0:kc, ci, g, :], hps[g2][0:kc, :])

    # transpose two chunks ahead of the matmuls so the PSUM->SBUF copies
    # have a full chunk-period of slack
    xT2s = {0: emit_transposes(0, x16s[0]), 1: emit_transposes(1, x16s[1])}
    for c in range(NCHUNK):
        if c + XIO_BUFS - 1 < NCHUNK:
            x16s.append(load_chunk(c + XIO_BUFS - 1))
        if c + 2 < NCHUNK:
            xT2s[c + 2] = emit_transposes(c + 2, x16s[c + 2])
        emit_matmuls(c, xT2s.pop(c))

    if debug:
        for ci in range(3):
            nc.sync.dma_start(io["h_dbg"].ap()[ci:ci + 1, :, :], hT[ci][:])

    # ---------------- BN1 stats exchange ----------------
    # local aggregate per chunk, rebuild (count, mean, M2) triples, AllGather
    locmv = pp.tile([128, 3, 2], f32, tag="locmv", name="locmv")
    trip = pp.tile([128, 3, 3], f32, tag="trip", name="trip")
    nc.vector.memset(trip[:, :, 0:1], float(BC))
    for ci, (k0, kc, off) in enumerate(KCH):
        nc.vector.bn_aggr(locmv[0:kc, ci, :], bst[0:kc, ci, :, :])
        nc.vector.tensor_copy(trip[0:kc, ci, 1:2], locmv[0:kc, ci, 0:1])
        nc.vector.tensor_scalar_mul(trip[0:kc, ci, 2:3],
                                    locmv[0:kc, ci, 1:2], float(BC))

    if l1_only:
        nc.vector.memset(oT[:], 0.0)
        nc.sync.dma_start(io["out"].ap(), oT[:])
        return

    ag1_in = dram.tile([128, 9], f32, tag="ag1_in", name="ag1_in")
    ag1_out = dram.tile([ranks * 128, 9], f32, tag="ag1_out", name="ag1_out")
    nc.sync.dma_start(ag1_in[:], trip[:].rearrange("p a b -> p (a b)"))
    nc.gpsimd.collective_compute(
        "AllGather", ALU.bypass,
        replica_groups=[list(range(ranks))],
        ins=[ag1_in.opt()], outs=[ag1_out.opt()])
    allst1 = pp.tile([128, ranks, 3, 3], f32, tag="allst1", name="allst1")
    nc.sync.dma_start(
        allst1[:].rearrange("p r a b -> p r (a b)"),
        ag1_out.rearrange("(r p) c -> p r c", p=128))
    gst1 = pp.tile([128, 3, 2], f32, tag="gst1", name="gst1")
    for ci, (k0, kc, off) in enumerate(KCH):
        nc.vector.bn_aggr(gst1[0:kc, ci, :], allst1[0:kc, :, ci, :])

    # a1 = gamma1 * rsqrt(var + eps) = sqrt(recip(var+eps) * gamma1^2)
    a1 = pp.tile([128, 3], f32, tag="a1", name="a1")
    vtmp = pp.tile([128, 8], f32, tag="vtmp", name="vtmp")
    for ci, (k0, kc, off) in enumerate(KCH):
        v = vtmp[0:kc, 1:2]
        rcp = vtmp[0:kc, 3:4]
        nc.vector.tensor_scalar_add(v, gst1[0:kc, ci, 1:2], EPS)
        nc.vector.reciprocal(rcp, v)
        nc.scalar.activation(a1[0:kc, ci:ci + 1], rcp,
                             AF.Sqrt, scale=prm[0:kc, ci:ci + 1])

    w2aT = []
    for ci, (k0, kc, off) in enumerate(KCH):
        wa = pp.tile([128, O], f16, tag=f"w2aT{ci}", name=f"w2aT{ci}")
        nc.vector.tensor_scalar(
            wa[0:kc, :], w2w[0:kc, O * ci:O * (ci + 1)],
            a1[0:kc, ci:ci + 1], None, op0=ALU.mult)
        w2aT.append(wa)

    # ---------------- layer 2 (group blocks, ci outer) ------------------
    for gb in range(0, NGRP, GBLK):
        ops = [ps_o.tile([O, GW], f32, tag="hps", name="ops")
               for _ in range(GBLK)]
        for ci, (k0, kc, off) in enumerate(KCH):
            for gi in range(GBLK):
                g = gb + gi
                nc.tensor.matmul(
                    ops[gi][:], w2aT[ci][0:kc, :],
                    hT[ci][0:kc, GW * g:GW * (g + 1)],
                    start=(ci == 0), stop=(ci == 2))
        for gi in range(GBLK):
            g = gb + gi
            nc.scalar.copy(oT[:, GW * g:GW * (g + 1)], ops[gi][:])
            nc.vector.bn_stats(bst2[:, g, :], ops[gi][:])

    # ---------------- BN2 stats exchange (pre-aggregated) ----------------
    locmv2 = pp.tile([O, 2], f32, tag="locmv2", name="locmv2")
    trip2 = pp.tile([O, 3], f32, tag="trip2", name="trip2")
    nc.vector.memset(trip2[:, 0:1], float(BC))
    nc.vector.bn_aggr(locmv2[:], bst2[:])
    nc.vector.tensor_copy(trip2[:, 1:2], locmv2[:, 0:1])
    nc.vector.tensor_scalar_mul(trip2[:, 2:3], locmv2[:, 1:2], float(BC))
    ag2_in = dram.tile([O, 3], f32, tag="ag2_in", name="ag2_in")
    ag2_out = dram.tile([ranks * O, 3], f32, tag="ag2_out", name="ag2_out")
    nc.sync.dma_start(ag2_in[:], trip2[:])
    nc.gpsimd.collective_compute(
        "AllGather", ALU.bypass,
        replica_groups=[list(range(ranks))],
        ins=[ag2_in.opt()], outs=[ag2_out.opt()])
    allst2 = pp.tile([O, ranks, 3], f32, tag="allst2", name="allst2")
    nc.sync.dma_start(
        allst2[:], ag2_out.rearrange("(r p) c -> p r c", p=O))
    gst2 = pp.tile([O, 2], f32, tag="gst2", name="gst2")
    nc.vector.bn_aggr(gst2[:], allst2[:])

    ab2 = pp.tile([O, 2], f32, tag="ab2", name="ab2")
    a2 = ab2[:, 0:1]
    b2 = ab2[:, 1:2]
    v2 = pp.tile([O, 6], f32, tag="v2tmp", name="v2tmp")
    nc.vector.tensor_scalar_add(v2[:, 1:2], gst2[:, 1:2], EPS)
    nc.vector.reciprocal(v2[:, 3:4], v2[:, 1:2])
    nc.scalar.activation(a2[:], v2[:, 3:4], AF.Sqrt, scale=prm[0:O, 3:4])
    nc.vector.tensor_mul(v2[:, 5:6], gst2[:, 0:1], a2[:])
    nc.vector.tensor_sub(b2[:], prm[0:O, 4:5], v2[:, 5:6])

    # ---------------- final affine in [10, B] layout + direct store -----
    # out = oT * a2 + b2 (per-partition scalars); Pool is ~2.4x slower
    # than DVE here, so split ~30/70.
    hb = (BC * 3 // 10) // 64 * 64
    nc.gpsimd.tensor_scalar(oT[:, 0:hb], oT[:, 0:hb], a2, b2,
                            op0=ALU.mult, op1=ALU.add)
    nc.vector.tensor_scalar(oT[:, hb:BC], oT[:, hb:BC], a2, b2,
                            op0=ALU.mult, op1=ALU.add)
    # Store [10, 8192] fp32 directly (10 contiguous 32KB runs); the host
    # transposes and unpermutes rows afterwards.
    nc.sync.dma_start(io["out"].ap(), oT[:])


def _build(debug=False, ranks=N_CORES, reps=1, l1_only=False):
    nc = bacc.Bacc("TRN2", target_bir_lowering=False, debug=False,
                   num_devices=ranks)

    io = {
        "x": nc.dram_tensor("x", [BC, DPAD], f16, kind="ExternalInput"),
        "w1bT": nc.dram_tensor("w1bT", [128, ND * WCOLS], f16,
                               kind="ExternalInput"),
        "w2bT": nc.dram_tensor("w2bT", [128, 3 * O], f16,
                               kind="ExternalInput"),
        "prm": nc.dram_tensor("prm", [128, 8], f32, kind="ExternalInput"),
        "out": nc.dram_tensor("out", [O, BC], f32, kind="ExternalOutput"),
    }
    if debug:
        io["h_dbg"] = nc.dram_tensor("h_dbg", [3, 128, NGRP * GW], f16,
                                     kind="ExternalOutput")

    with tile.TileContext(nc) as tc:
        with tc.tile_pool(name="persist", bufs=1) as pp, \
             tc.tile_pool(name="xio", bufs=XIO_BUFS) as xio, \
             tc.tile_pool(name="xTp", bufs=3) as xTp, \
             tc.tile_pool(name="ps_h", bufs=4, space="PSUM") as ps_h, \
             tc.tile_pool(name="ps_t", bufs=4, space="PSUM") as ps_t, \
             tc.tile_pool(name="dram", bufs=1, space="DRAM") as dram:
            P = dict(pp=pp, xio=xio, xTp=xTp,
                     ps_h=ps_h, ps_t=ps_t, dram=dram)
            for _ in range(reps):
                _emit(nc, tc, io, P, ranks, debug, l1_only)

    nc.compile()
    return nc


_CACHE = {}


def get_nc(debug=False, ranks=N_CORES, reps=1, l1_only=False):
    key = (debug, ranks, reps, l1_only)
    if key not in _CACHE:
        _CACHE[key] = _build(debug, ranks, reps, l1_only)
    return _CACHE[key]


def _row_perm():
    """Logical row index for oT column b.

    SBUF batch index b = 1024*chunk + 128*g + c maps to logical row
    1024*chunk + 8*c + g.
    """
    b = np.arange(BC)
    return 1024 * (b // 1024) + 8 * (b % 128) + (b % 1024) // 128


_PERM = _row_perm()


def _pack_weights(W1, W2, gamma1, gamma2, beta2):
    W1 = np.asarray(W1, dtype=np.float32)
    W2 = np.asarray(W2, dtype=np.float32)
    w1b = np.where(W1 >= 0, 1.0, -1.0).astype(np.float16)   # [300, 784]
    w1p = np.zeros((128, ND * WCOLS), dtype=np.float16)
    w1v = w1p.reshape(128, ND, WCOLS)
    w2b = np.where(W2 >= 0, 1.0, -1.0).astype(np.float16)   # [10, 300]
    w2p = np.zeros((128, 3 * O), dtype=np.float16)
    for ci, (k0, kc, off) in enumerate(KCH):
        wt = w1b[k0:k0 + kc, :].T                            # [784, kc]
        wtp = np.zeros((DPAD, kc), dtype=np.float16)
        wtp[0:D] = wt
        w1v[:, :, off:off + kc] = \
            wtp.reshape(ND, 128, kc).transpose(1, 0, 2)
        w2p[0:kc, O * ci:O * (ci + 1)] = w2b[:, k0:k0 + kc].T
    g1 = np.asarray(gamma1, dtype=np.float32).reshape(H)
    g2 = np.asarray(gamma2, dtype=np.float32).reshape(O)
    b2 = np.asarray(beta2, dtype=np.float32).reshape(O)
    prm = np.zeros((128, 8), dtype=np.float32)
    for ci, (k0, kc, off) in enumerate(KCH):
        prm[0:kc, ci] = g1[k0:k0 + kc] ** 2
    prm[0:O, 3] = g2 ** 2
    prm[0:O, 4] = b2
    return w1p, w2p, prm


def make_in_maps(x, W1, gamma1, W2, gamma2, beta2, ranks=N_CORES):
    x = np.asarray(x, dtype=np.float32)
    xp = np.zeros((B_FULL, DPAD), dtype=np.float16)
    xp[:, 0:D] = x
    w1p, w2p, prm = _pack_weights(W1, W2, gamma1, gamma2, beta2)
    return [{
        "x": xp[c * BC:(c + 1) * BC],
        "w1bT": w1p, "w2bT": w2p, "prm": prm,
    } for c in range(ranks)]


def gather_out(res, ranks=N_CORES):
    """Transpose the [10, 8192] store and undo the slab permutation."""
    outs = []
    for c in range(ranks):
        d = res.results[c]["out"]           # [O, BC]
        o = np.empty((BC, O), dtype=d.dtype)
        o[_PERM] = d.T
        outs.append(o)
    return np.concatenate(outs, axis=0)


def kernel(x, W1, gamma1, beta1, W2, gamma2, beta2):
    nc = get_nc()
    in_maps = make_in_maps(x, W1, gamma1, W2, gamma2, beta2)
    res = bass_utils.run_bass_kernel_spmd(
        nc, in_maps, core_ids=list(range(N_CORES)))
    return gather_out(res)
